# revision 1
# baseline (speedup 1.0000x reference)
"""Sliding-window causal self-attention (GQA + RoPE + QK-RMSnorm + gated
value-embedding) for Trainium2, SPMD over 8 NeuronCores.

Sharding: core c = (b, g) with b = c // 4 (batch), g = c % 4 (kv head group).
Each core computes its 4 query heads / 1 kv head for its batch and produces a
partial output projection [T, E]; the host sums the 4 partials per batch
(tensor-parallel all-reduce done host-side).

Layout strategy inside a core (T=2048, E=1024, D=64, window=1024):
 - host passes xT = x[b].T so every matmul contraction (over E) has E on
   partitions.
 - q/k are produced directly in [head-dim, T] layout ("qT"): lhsT = W.T
   chunks, rhs = xT chunks.  RoPE's half-swap is done with SBUF->SBUF DMA
   partition moves; cos/sin arrive pre-tiled as [128, T] patterns.  RMS-norm
   sums over head-dim (partitions) via a matmul with a block-indicator
   matrix, rsqrt = DVE reciprocal + ACT sqrt, broadcast back over partitions
   with stride-0 partition DMA.
 - scores are computed transposed, sT[k, q] = kT.T @ qT, so softmax'd
   probabilities p[k, q] feed the PV matmul directly as the moving operand:
   outT[d, q] = v_aug[k, d+1].T @ p[k, q].  v carries an appended
   ones-column, so outT row 64 accumulates the softmax denominator for free.
 - exp on ScalarE with the 1/sqrt(D) folded into the activation scale;
   causal/window edge tiles are masked by multiplying with precomputed 0/1
   bf16 tiles after exp.
 - all matmul operands are bf16 (fp32 PSUM accumulate).
"""

import numpy as np
import ml_dtypes

import concourse.bass as bass
import concourse.mybir as mybir
import concourse.tile as tile
from concourse import bacc
from concourse.bass_utils import run_bass_kernel_spmd

BF16 = mybir.dt.bfloat16
F32 = mybir.dt.float32
BF = ml_dtypes.bfloat16

T, E, H, HKV, D = 2048, 1024, 16, 4, 64
WIN = 1024
NQH = 4            # q heads per core
NQ = NQH * D       # 256 q dims per core
EPS = 1.1920929e-07
DLIST = [0, 128, 256, 384, -1024, -896, -768, -640]  # partial-mask offsets
DEBUG_STOP_AFTER = None  # None | 'C' | 'D' : truncate emission for debugging


def _chunks_for(qc):
    """k-chunk (128-wide) absolute indices needed for q rows [512qc, 512qc+512)."""
    return list(range(max(0, 4 * qc - 8), 4 * qc + 4))


def emit_body(nc, tc, dram, pools):
    const, big, work, psmisc, psscore, dpool = pools
    MUL = mybir.AluOpType.mult
    ADD = mybir.AluOpType.add
    AF = mybir.ActivationFunctionType

    # ---- load constants / inputs to SBUF (single batched DMAs) ----
    wqkg_all = big.tile([128, 8, 321], BF16, tag="wqkg", name="wqkg")
    nc.scalar.dma_start(wqkg_all[:],
                        dram["wqkg"].rearrange("(e p) m -> p e m", p=128))
    wqkg_sb = [wqkg_all[:, i, :] for i in range(8)]
    xT_all = big.tile([128, 8, T], BF16, tag="xT", name="xT")
    xTr = dram["xT"].rearrange("(e p) t -> p e t", p=128)
    for i in range(8):
        eng = nc.sync if i % 2 == 0 else nc.scalar
        eng.dma_start(xT_all[:, i:i + 1, :], xTr[:, i:i + 1, :])
    xT_sb = [xT_all[:, i, :] for i in range(8)]
    wv_all = big.tile([128, 8, 64], BF16, tag="wv", name="wv")
    nc.scalar.dma_start(wv_all[:],
                        dram["wv"].rearrange("(e p) m -> p e m", p=128))
    wv_sb = [wv_all[:, i, :] for i in range(8)]
    wp_all = big.tile([128, 2, 1024], BF16, tag="wp", name="wp")
    nc.scalar.dma_start(wp_all[:],
                        dram["wp"].rearrange("(e p) m -> p e m", p=128))
    wp_sb = [wp_all[:, i, :] for i in range(2)]
    crep = big.tile([128, T], BF16, tag="crep", name="crep")
    nc.sync.dma_start(crep[:], dram["crep"][:])
    srep = big.tile([128, T], BF16, tag="srep", name="srep")
    nc.sync.dma_start(srep[:], dram["srep"][:])
    vet2 = big.tile([64, T], BF16, tag="vet2", name="vet2")
    nc.scalar.dma_start(vet2[:], dram["vet2"][:])
    bind = big.tile([128, 2], BF16, tag="bind", name="bind")
    nc.scalar.dma_start(bind[:], dram["bind"][:])
    bw = big.tile([96, 448], BF16, tag="bw", name="bw")
    nc.scalar.dma_start(bw[:], dram["bw"][:])
    psw = big.tile([128, 128], BF16, tag="psw", name="psw")
    nc.scalar.dma_start(psw[:], dram["psw"][:])
    masks = big.tile([128, 8, 512], BF16, tag="masks", name="masks")
    nc.scalar.dma_start(masks[:], dram["masks"].rearrange("m p c -> p m c"))

    # ---- phase B: q/k/gate projections -> qT layout ----
    # raw (pre-rope) tiles: q01 rows h0d0..h0d63,h1d0..h1d63 ; q23 ; kg rows
    # 0-63 = k dims, row 64 = gate logits
    q_raw = [big.tile([128, T], BF16, tag=f"qraw{i}", name=f"qraw{i}") for i in range(2)]
    kg_raw = big.tile([65, T], BF16, tag="kgraw", name="kgraw")
    sig = big.tile([1, T], BF16, tag="sig", name="sig")
    vt_fin = big.tile([80, T], BF16, tag="vtfin", name="vtfin")
    nc.vector.memset(vt_fin[64:80, :], 0.0)
    nc.vector.memset(vt_fin[64:65, :], 1.0)
    v_sb = big.tile([128, 16, 80], BF16, tag="vsb", name="vsb")
    mslices = [(0, 128), (128, 128), (256, 65)]
    for tc4 in range(4):
        cs = slice(512 * tc4, 512 * (tc4 + 1))
        for mi, (moff, msz) in enumerate(mslices):
            ps = psmisc.tile([128, 512], F32, tag="misc", name="misc")
            for e in range(8):
                nc.tensor.matmul(
                    ps[:msz, :],
                    wqkg_sb[e][:, moff:moff + msz],
                    xT_sb[e][:, cs],
                    start=(e == 0), stop=(e == 7),
                )
            dest = q_raw[mi] if mi < 2 else kg_raw
            nc.scalar.copy(dest[:msz, cs], ps[:msz, :])
        # v projection + value-embedding gating for this chunk
        nc.scalar.activation(sig[:, cs], kg_raw[64:65, cs], AF.Sigmoid)
        vps = psmisc.tile([128, 512], F32, tag="misc", name="vps")
        for e in range(8):
            nc.tensor.matmul(vps[:64, :], wv_sb[e][:], xT_sb[e][:, cs],
                             start=(e == 0), stop=(e == 7))
        gps = psmisc.tile([128, 512], F32, tag="misc", name="gps")
        nc.tensor.matmul(gps[:64, :], bw[0:1, 384:448], sig[:, cs],
                         start=True, stop=True)
        tmp = work.tile([64, 512], BF16, tag="vtmp", name="vtmp")
        nc.vector.tensor_mul(tmp[:], gps[:64, :], vet2[:, cs])
        nc.vector.tensor_add(vt_fin[0:64, cs], vps[:64, :], tmp[:])
        for kk in range(4):
            ka = 4 * tc4 + kk
            nc.scalar.dma_start_transpose(v_sb[:, ka, 0:80],
                                          vt_fin[:, 128 * ka:128 * (ka + 1)])

    # ---- phase C: rope + rms-norm ----
    # RoPE half-swap via permutation matmul on PE (no DMA round trip)
    rot_q = [big.tile([128, T], BF16, tag=f"rotq{i}", name=f"rotq{i}") for i in range(2)]
    rot_k = big.tile([64, T], BF16, tag="rotk", name="rotk")
    for tc4 in range(4):
        cs = slice(512 * tc4, 512 * (tc4 + 1))
        for i in range(2):
            swp = psmisc.tile([128, 512], F32, tag="misc", name="swp")
            nc.tensor.matmul(swp[:], psw[:], q_raw[i][:, cs],
                             start=True, stop=True)
            t2 = work.tile([128, 512], BF16, tag="ropet2", name="ropet2", bufs=3)
            nc.vector.tensor_mul(rot_q[i][:, cs], q_raw[i][:, cs], crep[:, cs])
            nc.vector.tensor_mul(t2[:], swp[:], srep[:, cs])
            nc.vector.tensor_add(rot_q[i][:, cs], rot_q[i][:, cs], t2[:])
        swp = psmisc.tile([128, 512], F32, tag="misc", name="swp")
        nc.tensor.matmul(swp[:64, :], psw[0:64, 0:64], kg_raw[0:64, cs],
                         start=True, stop=True)
        t2 = work.tile([128, 512], BF16, tag="ropet2", name="ropet2", bufs=3)
        nc.vector.tensor_mul(rot_k[:, cs], kg_raw[0:64, cs], crep[0:64, cs])
        nc.vector.tensor_mul(t2[:64, :], swp[:64, :], srep[0:64, cs])
        nc.vector.tensor_add(rot_k[:, cs], rot_k[:, cs], t2[:64, :])

    # sum of squares over each 64-dim head block (partition dim) via matmul
    sq_q = [big.tile([128, T], BF16, tag=f"sq{i}", name=f"sq{i}") for i in range(2)]
    sq_k = big.tile([64, T], BF16, tag="sqk", name="sqk")
    for tc4 in range(4):
        cs = slice(512 * tc4, 512 * (tc4 + 1))
        for i in range(2):
            nc.vector.tensor_mul(sq_q[i][:, cs], rot_q[i][:, cs],
                                 rot_q[i][:, cs])
        nc.vector.tensor_mul(sq_k[:, cs], rot_k[:, cs], rot_k[:, cs])

    # engine ops need 32-aligned partition bases: groups at rows 0/32/64
    rstd_raw = big.tile([96, T], F32, tag="rstdraw", name="rstdraw")
    rstd = big.tile([96, T], BF16, tag="rstd", name="rstd")
    # rows outside the written groups feed the indicator matmul multiplied by
    # zero; memset so they can't carry NaN bit patterns
    nc.vector.memset(rstd[:], 0.0)
    qn = q_raw           # q_raw dead once rope is done; reuse slots
    kn_dup = sq_q[0]     # sq scratch dead after the sumsq matmuls
    srcs = [(sq_q[0], 128, 2, 0), (sq_q[1], 128, 2, 32), (sq_k, 64, 1, 64)]
    for tc4 in range(4):
        cs = slice(512 * tc4, 512 * (tc4 + 1))
        for (src, pp, nb, row0) in srcs:
            ps = psmisc.tile([128, 512], F32, tag="misc", name="misc")
            nc.tensor.matmul(ps[:nb, :], bind[:pp, :nb], src[:pp, cs],
                             start=True, stop=True)
            # mean + eps
            nc.vector.tensor_scalar(rstd_raw[row0:row0 + nb, cs], ps[:nb, :],
                                    1.0 / 64.0, EPS, MUL, ADD)
        for row0, nb in [(0, 2), (32, 2), (64, 1)]:
            nc.vector.reciprocal(rstd_raw[row0:row0 + nb, cs],
                                 rstd_raw[row0:row0 + nb, cs])
            nc.scalar.activation(rstd[row0:row0 + nb, cs],
                                 rstd_raw[row0:row0 + nb, cs], AF.Sqrt)
        # broadcast rstd across head-dim partitions via indicator matmuls
        # (bw cols: [0:128) tile0, [128:256) tile1, [256:320) k, [384:448) ones)
        for ti in range(2):
            bps = psmisc.tile([128, 512], F32, tag="misc", name="bps")
            nc.tensor.matmul(bps[:], bw[:, 128 * ti:128 * (ti + 1)],
                             rstd[:, cs], start=True, stop=True)
            nc.vector.tensor_mul(qn[ti][:, cs], rot_q[ti][:, cs], bps[:])
        bps = psmisc.tile([128, 512], F32, tag="misc", name="bps")
        nc.tensor.matmul(bps[:64, :], bw[:, 256:320], rstd[:, cs],
                         start=True, stop=True)
        nc.vector.tensor_mul(kn_dup[0:64, cs], rot_k[:, cs], bps[:64, :])
        nc.sync.dma_start(kn_dup[64:128, cs], kn_dup[0:64, cs])

    if DEBUG_STOP_AFTER == 'D':
        return
    # ---- phase E: attention ----
    yT = rot_q           # rot_q dead after the rstd multiply; reuse slots
    for qc in range(4):
        t0 = 512 * qc
        qs = slice(t0, t0 + 512)
        kas = _chunks_for(qc)
        # unnormalized PV output + denominator reciprocal collectors
        yu = work.tile([65, 4, 512], BF16, tag="yu", name="yu", bufs=2)
        rec = work.tile([1, 4, 512], F32, tag="rec", name="rec", bufs=2)
        recb = work.tile([1, 4, 512], BF16, tag="recb16", name="recb16", bufs=2)
        for h in range(NQH):
            ti, base = h // 2, 64 * (h % 2)
            pv = psmisc.tile([128, 512], F32, tag="misc", name="pv")
            npair = len(kas) // 2
            for pr in range(npair):
                s_ps = psscore.tile([128, 1024], F32, tag="s", name="s")
                p_sb = work.tile([128, 1024], BF16, tag="p", name="p", bufs=6)
                for j in range(2):
                    ka = kas[2 * pr + j]
                    nc.tensor.matmul(
                        s_ps[:, 512 * j:512 * (j + 1)],
                        kn_dup[base:base + 64, 128 * ka:128 * (ka + 1)],
                        qn[ti][base:base + 64, qs],
                        start=True, stop=True,
                    )
                nc.scalar.activation(p_sb[:], s_ps[:], AF.Exp, scale=0.125)
                d0 = 128 * kas[2 * pr] - t0
                d1 = 128 * kas[2 * pr + 1] - t0
                m0 = DLIST.index(d0) if (d0 >= 0 or d0 <= -640) else None
                m1 = DLIST.index(d1) if (d1 >= 0 or d1 <= -640) else None
                if m0 is not None and m1 == m0 + 1 and m0 % 2 == 0:
                    # adjacent mask pair: one wide multiply
                    nc.vector.tensor_mul(p_sb[:], p_sb[:],
                                         masks[:, m0:m0 + 2, :])
                else:
                    for j, m in ((0, m0), (1, m1)):
                        if m is not None:
                            nc.vector.tensor_mul(
                                p_sb[:, 512 * j:512 * (j + 1)],
                                p_sb[:, 512 * j:512 * (j + 1)],
                                masks[:, m, :])
                for j in range(2):
                    gi = 2 * pr + j
                    ka = kas[gi]
                    nc.tensor.matmul(pv[:65, :], v_sb[:, ka, 0:65],
                                     p_sb[:, 512 * j:512 * (j + 1)],
                                     start=(gi == 0), stop=(gi == len(kas) - 1))
            nc.any.tensor_copy(yu[:, h, :], pv[0:65, :])
            nc.vector.reciprocal(rec[:, h, :], pv[64:65, :])
        nc.any.tensor_copy(recb[:], rec[:])
        # broadcast 1/denom across the 64 head dims via a K=1 ones matmul
        for h in range(NQH):
            ti, base = h // 2, 64 * (h % 2)
            rbp = psmisc.tile([128, 512], F32, tag="misc", name="rbp")
            nc.tensor.matmul(rbp[:64, :], bw[0:1, 384:448], recb[:, h, :],
                             start=True, stop=True)
            nc.vector.tensor_mul(yT[ti][base:base + 64, qs],
                                 yu[0:64, h, :], rbp[:64, :])

    # ---- phase F: output projection ----
    for tt in range(16):
        ts_ = slice(128 * tt, 128 * (tt + 1))
        ob = work.tile([128, 1024], F32, tag="ob", name="ob")
        ps = psscore.tile([128, 1024], F32, tag="s", name="s")
        for nch in range(2):
            for ti in range(2):
                nc.tensor.matmul(ps[:, 512 * nch:512 * (nch + 1)],
                                 yT[ti][:, ts_],
                                 wp_sb[ti][:, 512 * nch:512 * (nch + 1)],
                                 start=(ti == 0), stop=(ti == 1))
        nc.any.tensor_copy(ob[:], ps[:])
        eng = nc.sync if tt % 2 == 0 else nc.scalar
        eng.dma_start(dram["out"][ts_, :], ob[:])


def build_nc(n_reps=1):
    nc = bacc.Bacc("TRN2", target_bir_lowering=False, debug=False)
    dram = {
        "xT": nc.dram_tensor("xT", [E, T], BF16, kind="ExternalInput"),
        "wqkg": nc.dram_tensor("wqkg", [E, 321], BF16, kind="ExternalInput"),
        "wv": nc.dram_tensor("wv", [E, 64], BF16, kind="ExternalInput"),
        "wp": nc.dram_tensor("wp", [NQ, E], BF16, kind="ExternalInput"),
        "crep": nc.dram_tensor("crep", [128, T], BF16, kind="ExternalInput"),
        "srep": nc.dram_tensor("srep", [128, T], BF16, kind="ExternalInput"),
        "vet2": nc.dram_tensor("vet2", [64, T], BF16, kind="ExternalInput"),
        "bind": nc.dram_tensor("bind", [128, 2], BF16, kind="ExternalInput"),
        "bw": nc.dram_tensor("bw", [96, 448], BF16, kind="ExternalInput"),
        "psw": nc.dram_tensor("psw", [128, 128], BF16, kind="ExternalInput"),
        "masks": nc.dram_tensor("masks", [8, 128, 512], BF16,
                                kind="ExternalInput"),
        "out": nc.dram_tensor("out", [T, E], F32, kind="ExternalOutput"),
    }
    with tile.TileContext(nc) as tc:
        with (
            tc.tile_pool(name="const", bufs=1) as const,
            tc.tile_pool(name="big", bufs=1) as big,
            tc.tile_pool(name="work", bufs=3) as work,
            tc.tile_pool(name="psmisc", bufs=4, space=bass.MemorySpace.PSUM) as psmisc,
            tc.tile_pool(name="psscore", bufs=2, space=bass.MemorySpace.PSUM) as psscore,
            tc.tile_pool(name="dpool", bufs=1, space=bass.MemorySpace.DRAM) as dpool,
        ):
            pools = (const, big, work, psmisc, psscore, dpool)
            for _ in range(n_reps):
                emit_body(nc, tc, dram, pools)
    nc.compile()
    return nc


def prep_inputs(x, ve, cos, sin, Wq, Wk, Wv, Wproj, Wgate):
    """Host-side sharding/layout prep -> list of 8 per-core input dicts."""
    x = np.asarray(x, np.float32)
    ve = np.asarray(ve, np.float32)
    cos = np.asarray(cos, np.float32).reshape(T, D // 2)
    sin = np.asarray(sin, np.float32).reshape(T, D // 2)
    Wq = np.asarray(Wq, np.float32)
    Wk = np.asarray(Wk, np.float32)
    Wv = np.asarray(Wv, np.float32)
    Wproj = np.asarray(Wproj, np.float32)
    Wgate = np.asarray(Wgate, np.float32)

    cT = np.ascontiguousarray(cos.T)          # [32, T]
    sT = np.ascontiguousarray(sin.T)
    crep = np.tile(cT, (4, 1)).astype(BF)      # [128, T]
    srep = np.tile(np.concatenate([sT, -sT], 0), (2, 1)).astype(BF)

    bind = np.zeros((128, 2), BF)
    bind[0:64, 0] = 1.0
    bind[64:128, 1] = 1.0

    bw = np.zeros((96, 448), BF)
    bw[0, 0:64] = 1.0       # tile0 head0 <- rstd row 0
    bw[1, 64:128] = 1.0     # tile0 head1 <- rstd row 1
    bw[32, 128:192] = 1.0   # tile1 head2 <- rstd row 32
    bw[33, 192:256] = 1.0   # tile1 head3 <- rstd row 33
    bw[64, 256:320] = 1.0   # k broadcast <- rstd row 64
    bw[0, 384:448] = 1.0    # ones row for gate broadcast

    psw = np.zeros((128, 128), BF)   # RoPE half-swap permutation per 64-block
    for blk in range(2):
        for d in range(32):
            psw[blk * 64 + 32 + d, blk * 64 + d] = 1.0
            psw[blk * 64 + d, blk * 64 + 32 + d] = 1.0

    j = np.arange(128)[:, None]
    r = np.arange(512)[None, :]
    masks = np.zeros((8, 128, 512), BF)
    for m, d in enumerate(DLIST):
        keep = ((d + j) <= r) & ((d + j) >= (r - WIN))
        masks[m] = keep.astype(BF)

    ins = []
    for c in range(8):
        b, g = c // 4, c % 4
        wgate_pad = np.zeros((E, 1), np.float32)
        wgate_pad[0:32, 0] = Wgate[g]
        wqkg = np.concatenate(
            [Wq[NQ * g:NQ * (g + 1)].T, Wk[D * g:D * (g + 1)].T, wgate_pad], axis=1
        )
        ins.append({
            "xT": np.ascontiguousarray(x[b].T).astype(BF),
            "wqkg": wqkg.astype(BF),
            "wv": np.ascontiguousarray(Wv[D * g:D * (g + 1)].T).astype(BF),
            "wp": np.ascontiguousarray(Wproj[:, NQ * g:NQ * (g + 1)].T).astype(BF),
            "crep": crep,
            "srep": srep,
            "vet2": np.ascontiguousarray(
                2.0 * ve[b, :, D * g:D * (g + 1)].T).astype(BF),
            "bind": bind,
            "bw": bw,
            "psw": psw,
            "masks": masks,
        })
    return ins


_NC_CACHE = {}


def _get_nc(n_reps=1):
    if n_reps not in _NC_CACHE:
        _NC_CACHE[n_reps] = build_nc(n_reps)
    return _NC_CACHE[n_reps]


def kernel(x, ve, cos, sin, Wq, Wk, Wv, Wproj, Wgate, window_size=1024):
    assert int(window_size) == WIN, f"kernel hardcodes window={WIN}"
    ins = prep_inputs(x, ve, cos, sin, Wq, Wk, Wv, Wproj, Wgate)
    nc = _get_nc(1)
    res = run_bass_kernel_spmd(nc, ins, list(range(8)))
    out = np.zeros((2, T, E), np.float32)
    for c in range(8):
        out[c // 4] += res.results[c]["out"]
    return out



# revision 40
# speedup vs baseline: 2.6291x; 2.6291x over previous
"""Sliding-window causal self-attention (GQA + RoPE + QK-RMSnorm + gated
value-embedding) for Trainium2, SPMD over 8 NeuronCores.

Sharding: core c = (b, g) with b = c // 4 (batch), g = c % 4 (kv head group).
Each core computes its 4 query heads / 1 kv head for its batch and produces a
partial output projection [T, E] (bf16); the host sums the 4 partials per
batch (tensor-parallel all-reduce done host-side).

Key structure (v2):
 - xT arrives tc4-major ([4, 128, 8*512]) so each 512-token chunk is one
   128-descriptor DMA; weights/constants packed into few DMAs on pool/sync
   queues.
 - gate logits (K=32 matmul) + tanh run up front: 2*sigmoid(u) = 1+tanh(u/2)
   keeps ACT on the exp_and_others table set; ve is added into the V psum
   via an identity-matmul accumulate.
 - attention processes kv chunks at restricted q-width (diagonal chunk at
   offset d covers q in [d, 512); window-tail chunk covers q in [0, d+1152)),
   cutting score/exp/PV work ~25%.
 - causal/window boundary masking is ADDITIVE via PE: identity-stationary
   matmuls accumulate a [128,128] 0/-30000 triangle tile into the scores
   psum before exp (no DVE mask multiplies).
 - scores for a head PAIR (rows 0-63 / 64-127 of qn) are packed into one
   [128, 2, 512] psum tile; several chunks pack along the free dim; ONE exp
   per group covers both heads. v carries an appended ones-column so PV row
   64 accumulates the softmax denominator for free.
 - all matmul operands bf16 (fp32 PSUM accumulate); output stored bf16.
"""

import numpy as np
import ml_dtypes

import concourse.bass as bass
import concourse.mybir as mybir
import concourse.tile as tile
from concourse import bacc
from concourse.bass_utils import run_bass_kernel_spmd

BF16 = mybir.dt.bfloat16
F32 = mybir.dt.float32
BF = ml_dtypes.bfloat16

T, E, H, HKV, D = 2048, 1024, 16, 4, 64
WIN = 1024
NQH = 4            # q heads per core
NQ = NQH * D       # 256 q dims per core
EPS = 1.1920929e-07
NEG = -30000.0

# packed-constant free-dim offsets (cpack: small consts; csrep: cos/sin)
C_PSW = 0
C_ID = 128
C_TRID = 256
C_TRIT = 384
C_BIND = 512
C_BW = 514
C_EPS = 962
C_Z65 = 1027
C_TOT = 1092


def chunk_meta(qc):
    """(ka, q_lo, q_hi, kind) for kv chunks needed by q rows [512qc, 512qc+512).

    kind: 'full' (no mask), 'diag' (mask block at cols [q_lo, q_lo+128)),
    'tail' (mask block at cols [q_hi-128, q_hi)). Ordered full-width first so
    the first PV matmul (start=True) covers the whole [0,512) free range.
    """
    t0 = 512 * qc
    out = []
    for ka in range(max(0, 4 * qc - 8), 4 * qc + 4):
        d = 128 * ka - t0
        if d >= 0:
            out.append((ka, d, 512, 'diag'))
        elif d <= -640:
            out.append((ka, 0, d + 1152, 'tail'))
        else:
            out.append((ka, 0, 512, 'full'))
    out.sort(key=lambda c: c[2] - c[1], reverse=True)
    return out


def pack_groups(chunks, cap=512):
    """Greedy-pack chunks into groups with total width <= cap."""
    groups, cur, w = [], [], 0
    for c in chunks:
        cw = c[2] - c[1]
        if w + cw > cap and cur:
            groups.append(cur)
            cur, w = [], 0
        cur.append(c)
        w += cw
    if cur:
        groups.append(cur)
    return groups


def emit_body(nc, tc, dram, pools):
    const, big, work, psmisc, psscore, dpool = pools
    AF = mybir.ActivationFunctionType

    # ---- load inputs to SBUF (few, large DMAs; pool/sync queues) ----
    # pool: wqkg, xT0, wv, xT2 ; sync: cpack, xT1, vet, xT3, wp —
    # ordered so phase B(c4=0) can start as early as possible.
    wqkg_all = big.tile([128, 8, 321], BF16, tag="wqkg", name="wqkg")
    nc.gpsimd.dma_start(wqkg_all[:],
                        dram["wqkg"].rearrange("(e p) m -> p e m", p=128))
    wqkg_sb = [wqkg_all[:, i, :] for i in range(8)]
    cpack = big.tile([128, C_TOT], BF16, tag="cpack", name="cpack")
    nc.sync.dma_start(cpack[:], dram["cpack"][:])
    xT_all = big.tile([128, 4, 8, 512], BF16, tag="xT", name="xT")
    nc.gpsimd.dma_start(xT_all[:, 0, :, :],
                        dram["xT4"][0].rearrange("p (e t) -> p e t", e=8))
    wv_all = big.tile([128, 8, 64], BF16, tag="wv", name="wv")
    nc.gpsimd.dma_start(wv_all[:],
                        dram["wv"].rearrange("(e p) m -> p e m", p=128))
    wv_sb = [wv_all[:, i, :] for i in range(8)]
    nc.sync.dma_start(xT_all[:, 1, :, :],
                      dram["xT4"][1].rearrange("p (e t) -> p e t", e=8))
    vet = big.tile([64, T], BF16, tag="vet", name="vet")
    nc.sync.dma_start(vet[:], dram["vet"][:])
    nc.gpsimd.dma_start(xT_all[:, 2, :, :],
                        dram["xT4"][2].rearrange("p (e t) -> p e t", e=8))
    nc.sync.dma_start(xT_all[:, 3, :, :],
                      dram["xT4"][3].rearrange("p (e t) -> p e t", e=8))
    csrep = big.tile([128, 2, 2048], BF16, tag="csrep", name="csrep")
    nc.gpsimd.dma_start(csrep[:], dram["csrep"][:])
    wp_all = big.tile([128, 2, 1024], BF16, tag="wp", name="wp")
    nc.sync.dma_start(wp_all[:],
                      dram["wp"].rearrange("(e p) m -> p e m", p=128))
    wp_sb = [wp_all[:, i, :] for i in range(2)]

    crep = csrep[:, 0, :]
    srep = csrep[:, 1, :]
    psw = cpack[:, C_PSW:C_PSW + 128]
    id128 = cpack[:, C_ID:C_ID + 128]
    trid = cpack[:, C_TRID:C_TRID + 128]
    trit = cpack[:, C_TRIT:C_TRIT + 128]
    bind = cpack[:, C_BIND:C_BIND + 2]
    bw = cpack[0:65, C_BW:C_BW + 448]
    eps65 = cpack[64:65, C_EPS:C_EPS + 65]
    zero65 = cpack[64:65, C_Z65:C_Z65 + 65]

    def xT_sb(c4, e):
        return xT_all[:, c4, e, :]

    # ---- gate prologue: u = x[:, :32] @ Wgate.T ; th = tanh(u/2) ----
    th = big.tile([1, T], BF16, tag="th", name="th")
    for c4 in range(4):
        cs = slice(512 * c4, 512 * (c4 + 1))
        ups = psmisc.tile([128, 512], F32, tag="misc", name="ups")
        nc.tensor.matmul(ups[0:1, :], wqkg_sb[0][:, 320:321], xT_sb(c4, 0),
                         start=True, stop=True)
        nc.scalar.activation(th[:, cs], ups[0:1, :], AF.Tanh, scale=0.5)

    # ---- phase B: q/k/v projections -> transposed head-dim layout ----
    q_raw = [big.tile([128, T], BF16, tag=f"qraw{i}", name=f"qraw{i}")
             for i in range(2)]
    kg_raw = big.tile([64, T], BF16, tag="kgraw", name="kgraw")
    vt_fin = big.tile([80, T], BF16, tag="vtfin", name="vtfin")
    nc.vector.memset(vt_fin[64:80, :], 0.0)
    nc.vector.memset(vt_fin[64:65, :], 1.0)
    v_sb = big.tile([128, 16, 80], BF16, tag="vsb", name="vsb")
    mslices = [(0, 128), (128, 128), (256, 64)]
    for c4 in range(4):
        cs = slice(512 * c4, 512 * (c4 + 1))
        for mi, (moff, msz) in enumerate(mslices):
            ps = psmisc.tile([128, 512], F32, tag="misc", name="misc")
            for e in range(8):
                nc.tensor.matmul(
                    ps[:msz, :],
                    wqkg_sb[e][:, moff:moff + msz],
                    xT_sb(c4, e),
                    start=(e == 0), stop=(e == 7),
                )
            dest = q_raw[mi] if mi < 2 else kg_raw
            nc.scalar.copy(dest[:msz, cs], ps[:msz, :])
        # v projection + ve (identity-matmul accumulate) + tanh gating
        vps = psmisc.tile([128, 512], F32, tag="misc", name="vps")
        for e in range(8):
            nc.tensor.matmul(vps[:64, :], wv_sb[e][:], xT_sb(c4, e),
                             start=(e == 0), stop=False)
        nc.tensor.matmul(vps[:64, :], id128[0:64, 0:64], vet[:, cs],
                         start=False, stop=True)
        gps = psmisc.tile([128, 512], F32, tag="misc", name="gps")
        nc.tensor.matmul(gps[:64, :], bw[0:1, 384:448], th[:, cs],
                         start=True, stop=True)
        tmp = work.tile([64, 512], BF16, tag="vtmp", name="vtmp")
        nc.vector.tensor_mul(tmp[:], gps[:64, :], vet[:, cs])
        nc.vector.tensor_add(vt_fin[0:64, cs], vps[:64, :], tmp[:])
        for kk in range(4):
            ka = 4 * c4 + kk
            nc.scalar.dma_start_transpose(v_sb[:, ka, 0:80],
                                          vt_fin[:, 128 * ka:128 * (ka + 1)])

    # ---- phase C: rope + rms-norm (emitted per 512-chunk, interleaved
    # with phase E below so E's dense PE work hides C's serial chain) ----
    rot_q = [big.tile([128, T], BF16, tag=f"rotq{i}", name=f"rotq{i}")
             for i in range(2)]
    rot_k = big.tile([64, T], BF16, tag="rotk", name="rotk")
    sq_q = [big.tile([128, T], BF16, tag=f"sq{i}", name=f"sq{i}")
            for i in range(2)]
    sq_k = big.tile([64, T], BF16, tag="sqk", name="sqk")
    # rstd' = 1/sqrt(ss + 64*eps) at rows {0,1},{32,33},{64}; the missing
    # x8 (rstd = 8/sqrt(ss+64eps) = 1/sqrt(mean+eps)) is folded into bw.
    # The eps64 ones-matmul seeds the whole [0:65] row range so unused rows
    # reciprocal/sqrt to finite junk (bw rows there are zero).
    rstd_raw = big.tile([96, T], F32, tag="rstdraw", name="rstdraw")
    rstd = big.tile([96, T], BF16, tag="rstd", name="rstd")
    qn = q_raw           # q_raw dead once rope is done; reuse slots
    kn_dup = sq_q[0]     # sq scratch dead after the sumsq matmuls
    srcs = [(sq_q[0], 128, 2, 0), (sq_q[1], 128, 2, 32), (sq_k, 64, 1, 64)]

    def emit_C_rope(c4):
        cs = slice(512 * c4, 512 * (c4 + 1))
        for i in range(2):
            swp = psmisc.tile([128, 512], F32, tag="misc", name="swp")
            nc.tensor.matmul(swp[:], psw[:], q_raw[i][:, cs],
                             start=True, stop=True)
            swps = work.tile([128, 512], BF16, tag="swps", name="swps",
                             bufs=3)
            nc.scalar.copy(swps[:], swp[:])
            t2 = work.tile([128, 512], BF16, tag="ropet2", name="ropet2",
                           bufs=3)
            nc.vector.tensor_mul(rot_q[i][:, cs], q_raw[i][:, cs],
                                 crep[:, cs])
            nc.vector.tensor_mul(t2[:], swps[:], srep[:, cs])
            nc.gpsimd.tensor_add(rot_q[i][:, cs], rot_q[i][:, cs], t2[:])
            nc.gpsimd.tensor_mul(sq_q[i][:, cs], rot_q[i][:, cs],
                                 rot_q[i][:, cs])
        swp = psmisc.tile([128, 512], F32, tag="misc", name="swp")
        nc.tensor.matmul(swp[:64, :], psw[0:64, 0:64], kg_raw[0:64, cs],
                         start=True, stop=True)
        swps = work.tile([128, 512], BF16, tag="swps", name="swps", bufs=3)
        nc.scalar.copy(swps[:64, :], swp[:64, :])
        t2 = work.tile([128, 512], BF16, tag="ropet2", name="ropet2", bufs=3)
        nc.vector.tensor_mul(rot_k[:, cs], kg_raw[0:64, cs], crep[0:64, cs])
        nc.vector.tensor_mul(t2[:64, :], swps[:64, :], srep[0:64, cs])
        nc.gpsimd.tensor_add(rot_k[:, cs], rot_k[:, cs], t2[:64, :])
        nc.gpsimd.tensor_mul(sq_k[:, cs], rot_k[:, cs], rot_k[:, cs])

    def emit_C_rstd(c4):
        cs = slice(512 * c4, 512 * (c4 + 1))
        ps = psmisc.tile([128, 512], F32, tag="misc", name="misc")
        nc.tensor.matmul(ps[0:65, :], eps65, vt_fin[64:65, cs],
                         start=True, stop=False)
        for si, (src, pp, nb, row0) in enumerate(srcs):
            nc.tensor.matmul(ps[row0:row0 + nb, :], bind[:pp, :nb],
                             src[:pp, cs], start=False, stop=False)
        nc.tensor.matmul(ps[0:65, :], zero65, vt_fin[64:65, cs],
                         start=False, stop=True)
        nc.vector.reciprocal(rstd_raw[0:65, cs], ps[0:65, :])
        nc.scalar.activation(rstd[0:65, cs], rstd_raw[0:65, cs], AF.Sqrt)
        # broadcast rstd across head-dim partitions via indicator matmuls
        for ti in range(2):
            bps = psmisc.tile([128, 512], F32, tag="misc", name="bps")
            nc.tensor.matmul(bps[:], bw[:, 128 * ti:128 * (ti + 1)],
                             rstd[0:65, cs], start=True, stop=True)
            nc.vector.tensor_mul(qn[ti][:, cs], rot_q[ti][:, cs], bps[:])
        bps = psmisc.tile([128, 512], F32, tag="misc", name="bps")
        nc.tensor.matmul(bps[:64, :], bw[:, 256:320], rstd[0:65, cs],
                         start=True, stop=True)
        nc.vector.tensor_mul(kn_dup[0:64, cs], rot_k[:, cs], bps[:64, :])
        nc.gpsimd.dma_start(kn_dup[64:128, cs], kn_dup[0:64, cs])

    # ---- phases E+F software-pipelined per 512-q chunk ----
    # Emission order per qc: score/exp/PV groups for BOTH ti first, then
    # F(qc-1), then the normalize chains — so the PE queue never head-of-line
    # blocks on the rbp broadcast matmuls (their reciprocal inputs compute
    # during F(qc-1)'s matmuls).
    yT = rot_q           # rot_q dead after the rstd multiply; reuse slots

    def emit_F(qc):
        for half in range(2):
            ob2 = work.tile([128, 2, 1024], BF16, tag="ob2", name="ob2",
                            bufs=3)
            for k2 in range(2):
                tt = 4 * qc + 2 * half + k2
                ts_ = slice(128 * tt, 128 * (tt + 1))
                ps = psscore.tile([128, 2, 512], F32, tag="s", name="s")
                for nch in range(2):
                    for ti in range(2):
                        nc.tensor.matmul(
                            ps[:, nch, :],
                            yT[ti][:, ts_],
                            wp_sb[ti][:, 512 * nch:512 * (nch + 1)],
                            start=(ti == 0), stop=(ti == 1))
                if tt % 2 == 0:
                    nc.vector.tensor_copy(ob2[:, k2, :],
                                          ps.rearrange("p a b -> p (a b)"))
                else:
                    nc.scalar.copy(ob2[:, k2, :],
                                   ps.rearrange("p a b -> p (a b)"))
            eng = nc.gpsimd if half == 0 else nc.sync
            t0_ = 512 * qc + 256 * half
            eng.dma_start(
                dram["out"][t0_:t0_ + 256, :].rearrange(
                    "(t p) m -> p t m", p=128),
                ob2[:])

    def emit_E(qc):
        t0 = 512 * qc
        qs = slice(t0, t0 + 512)
        groups = pack_groups(chunk_meta(qc))
        pvs = {}
        for ti in range(2):
            pv = [psmisc.tile([128, 512], F32, tag="misc", name=f"pv{hb}")
                  for hb in range(2)]
            pvs[ti] = pv
            first = [True, True]
            nchunks = sum(len(g) for g in groups)
            done = 0

            def emit_pv(grp, offs, p_sb):
                nonlocal done, first
                for (ka, qlo, qhi, kind), (off_, w) in zip(grp, offs):
                    done += 1
                    for hb in range(2):
                        nc.tensor.matmul(
                            pv[hb][0:65, qlo:qhi],
                            v_sb[:, ka, 0:65],
                            p_sb[:, hb, off_:off_ + w],
                            start=first[hb], stop=(done == nchunks),
                        )
                    first = [False, False]

            prev = None
            for grp in groups:
                s_ps = psscore.tile([128, 2, 512], F32, tag="s", name="s")
                offs, off = [], 0
                for (ka, qlo, qhi, kind) in grp:
                    w = qhi - qlo
                    for hb, base in ((0, 0), (1, 64)):
                        nc.tensor.matmul(
                            s_ps[:, hb, off:off + w],
                            kn_dup[base:base + 64, 128 * ka:128 * (ka + 1)],
                            qn[ti][base:base + 64, t0 + qlo:t0 + qhi],
                            start=True, stop=(kind == 'full'),
                        )
                    if kind == 'diag':
                        for hb in range(2):
                            nc.tensor.matmul(s_ps[:, hb, off:off + 128],
                                             id128, trid,
                                             start=False, stop=True)
                    elif kind == 'tail':
                        mo = off + w - 128
                        for hb in range(2):
                            nc.tensor.matmul(s_ps[:, hb, mo:mo + 128],
                                             id128, trit,
                                             start=False, stop=True)
                    offs.append((off, w))
                    off += w
                # PV of the PREVIOUS group goes into the PE queue here, after
                # this group's score matmuls — so the PE never head-of-line
                # waits on this group's exp.
                if prev is not None:
                    emit_pv(*prev)
                p_sb = work.tile([128, 2, 512], BF16, tag="p", name="p",
                                 bufs=4)
                nc.scalar.activation(p_sb[:, :, 0:off], s_ps[:, :, 0:off],
                                     AF.Exp, scale=0.125)
                prev = (grp, offs, p_sb)
            emit_pv(*prev)

        if qc > 0:
            emit_F(qc - 1)

        # normalize: yT = pv[0:64] * (1/denom) broadcast over head dims
        for ti in range(2):
            pv = pvs[ti]
            yu = work.tile([64, 2, 512], BF16, tag="yu", name="yu", bufs=2)
            recb = work.tile([1, 2, 512], BF16, tag="recb", name="recb",
                             bufs=2)
            for hb in range(2):
                nc.vector.tensor_copy(yu[:, hb, :], pv[hb][0:64, :])
                with nc.allow_low_precision(
                        reason="1/denom broadcast operand is bf16 anyway"):
                    nc.vector.reciprocal(recb[:, hb, :], pv[hb][64:65, :])
            for hb in range(2):
                rbp = psmisc.tile([128, 512], F32, tag="misc", name="rbp")
                nc.tensor.matmul(rbp[:64, :], bw[0:1, 384:448],
                                 recb[:, hb, :], start=True, stop=True)
                nc.vector.tensor_mul(yT[ti][64 * hb:64 * hb + 64, qs],
                                     yu[:, hb, :], rbp[:64, :])

    for c4 in range(4):
        emit_C_rope(c4)
    for c4 in range(4):
        emit_C_rstd(c4)
    for qc in range(4):
        emit_E(qc)
    emit_F(3)


def build_nc(n_reps=1):
    nc = bacc.Bacc("TRN2", target_bir_lowering=False, debug=False)
    dram = {
        "xT4": nc.dram_tensor("xT4", [4, 128, 8 * 512], BF16,
                              kind="ExternalInput"),
        "wqkg": nc.dram_tensor("wqkg", [E, 321], BF16, kind="ExternalInput"),
        "wv": nc.dram_tensor("wv", [E, 64], BF16, kind="ExternalInput"),
        "wp": nc.dram_tensor("wp", [NQ, E], BF16, kind="ExternalInput"),
        "vet": nc.dram_tensor("vet", [64, T], BF16, kind="ExternalInput"),
        "cpack": nc.dram_tensor("cpack", [128, C_TOT], BF16,
                                kind="ExternalInput"),
        "csrep": nc.dram_tensor("csrep", [128, 2, 2048], BF16,
                                kind="ExternalInput"),
        "out": nc.dram_tensor("out", [T, E], BF16, kind="ExternalOutput"),
    }
    with tile.TileContext(nc) as tc:
        with (
            tc.tile_pool(name="const", bufs=1) as const,
            tc.tile_pool(name="big", bufs=1) as big,
            tc.tile_pool(name="work", bufs=3) as work,
            tc.tile_pool(name="psmisc", bufs=4,
                         space=bass.MemorySpace.PSUM) as psmisc,
            tc.tile_pool(name="psscore", bufs=2,
                         space=bass.MemorySpace.PSUM) as psscore,
            tc.tile_pool(name="dpool", bufs=1,
                         space=bass.MemorySpace.DRAM) as dpool,
        ):
            pools = (const, big, work, psmisc, psscore, dpool)
            for _ in range(n_reps):
                emit_body(nc, tc, dram, pools)
    nc.compile()
    return nc


def prep_inputs(x, ve, cos, sin, Wq, Wk, Wv, Wproj, Wgate):
    """Host-side sharding/layout prep -> list of 8 per-core input dicts."""
    x = np.asarray(x, np.float32)
    ve = np.asarray(ve, np.float32)
    cos = np.asarray(cos, np.float32).reshape(T, D // 2)
    sin = np.asarray(sin, np.float32).reshape(T, D // 2)
    Wq = np.asarray(Wq, np.float32)
    Wk = np.asarray(Wk, np.float32)
    Wv = np.asarray(Wv, np.float32)
    Wproj = np.asarray(Wproj, np.float32)
    Wgate = np.asarray(Wgate, np.float32)

    cT = np.ascontiguousarray(cos.T)          # [32, T]
    sT = np.ascontiguousarray(sin.T)
    crep = np.tile(cT, (4, 1)).astype(BF)      # [128, T]
    srep = np.tile(np.concatenate([sT, -sT], 0), (2, 1)).astype(BF)

    psw = np.zeros((128, 128), BF)   # RoPE half-swap permutation per 64-block
    for blk in range(2):
        for d in range(32):
            psw[blk * 64 + 32 + d, blk * 64 + d] = 1.0
            psw[blk * 64 + d, blk * 64 + 32 + d] = 1.0

    id128 = np.eye(128, dtype=BF)
    j = np.arange(128)[:, None]
    r = np.arange(128)[None, :]
    trid = np.where(r >= j, 0.0, NEG).astype(BF)   # diag: mask out r < j
    trit = np.where(r <= j, 0.0, NEG).astype(BF)   # tail: mask out r > j

    bind = np.zeros((128, 2), BF)
    bind[0:64, 0] = 1.0
    bind[64:128, 1] = 1.0

    bw = np.zeros((128, 448), BF)
    # 8.0 entries: rstd' rows hold 1/sqrt(ss+64eps); x8 gives 1/sqrt(mean+eps)
    bw[0, 0:64] = 8.0       # tile0 head0 <- rstd row 0
    bw[1, 64:128] = 8.0     # tile0 head1 <- rstd row 1
    bw[32, 128:192] = 8.0   # tile1 head2 <- rstd row 32
    bw[33, 192:256] = 8.0   # tile1 head3 <- rstd row 33
    bw[64, 256:320] = 8.0   # k broadcast <- rstd row 64
    bw[0, 384:448] = 1.0    # ones row for gate/denominator broadcast

    cpack = np.zeros((128, C_TOT), BF)
    cpack[64, C_EPS:C_EPS + 65] = 64.0 * EPS
    csrep = np.stack([crep, srep], axis=1)        # [128, 2, 2048]
    cpack[:, C_PSW:C_PSW + 128] = psw
    cpack[:, C_ID:C_ID + 128] = id128
    cpack[:, C_TRID:C_TRID + 128] = trid
    cpack[:, C_TRIT:C_TRIT + 128] = trit
    cpack[:, C_BIND:C_BIND + 2] = bind
    cpack[:, C_BW:C_BW + 448] = bw

    ins = []
    for c in range(8):
        b, g = c // 4, c % 4
        wgate_pad = np.zeros((E, 1), np.float32)
        wgate_pad[0:32, 0] = Wgate[g]
        wqkg = np.concatenate(
            [Wq[NQ * g:NQ * (g + 1)].T, Wk[D * g:D * (g + 1)].T, wgate_pad],
            axis=1)
        xt = np.ascontiguousarray(x[b].T).astype(BF)   # [E, T]
        # xT4[c4, p, e*512+t'] = x[b, 512*c4+t', 128e+p]
        xT4 = np.ascontiguousarray(
            xt.reshape(8, 128, 4, 512).transpose(2, 1, 0, 3).reshape(
                4, 128, 8 * 512))
        ins.append({
            "xT4": xT4,
            "wqkg": wqkg.astype(BF),
            "wv": np.ascontiguousarray(Wv[D * g:D * (g + 1)].T).astype(BF),
            "wp": np.ascontiguousarray(
                Wproj[:, NQ * g:NQ * (g + 1)].T).astype(BF),
            "vet": np.ascontiguousarray(
                ve[b, :, D * g:D * (g + 1)].T).astype(BF),
            "cpack": cpack,
            "csrep": csrep,
        })
    return ins


_NC_CACHE = {}


def _get_nc(n_reps=1):
    if n_reps not in _NC_CACHE:
        _NC_CACHE[n_reps] = build_nc(n_reps)
    return _NC_CACHE[n_reps]


def kernel(x, ve, cos, sin, Wq, Wk, Wv, Wproj, Wgate, window_size=1024):
    assert int(window_size) == WIN, f"kernel hardcodes window={WIN}"
    ins = prep_inputs(x, ve, cos, sin, Wq, Wk, Wv, Wproj, Wgate)
    nc = _get_nc(1)
    res = run_bass_kernel_spmd(nc, ins, list(range(8)))
    out = np.zeros((2, T, E), np.float32)
    for c in range(8):
        out[c // 4] += res.results[c]["out"].astype(np.float32)
    return out
